# revision 28
# baseline (speedup 1.0000x reference)
"""Trainium2 Bass kernel for nn_BondLenConstrain.

Contract: kernel(**inputs) takes the FULL (unsharded) inputs of
reference.setup_inputs() and returns the full [64, 4, 2048, 2] float32
resiEnergy tensor.  Data-parallel over the batch axis across 8 NeuronCores
(8 batches per core).

Host (numpy, indexing only): scatter atoms into dense residue grids exactly
like the reference, build the `todo` mask, gather the tiny per-residue-type
tables into per-residue coefficient planes (masked pairs get all-zero
coefficients -> device formula returns exactly 0), transpose coords to a
plane-contiguous blocked layout, and broadcast the (identical) nalt lanes
of the output.

Device math per residue pair (r-1, r):
    v2 = CA_r - N_r, v1 = C_{r-1} - N_r, v3 = CA_{r-1} - C_{r-1}
    bond  f1 = sqrt(d11) = exp(0.5 ln d11)
    angle theta = pi/2 - sign(c) * arctan(|c|/s),  s = sqrt(dxx*d11 - c^2)
          arctan over [0,inf) via  t' = arctan(exp(-|ln(|c|/s)|)) in [0,pi/4]
          (ScalarE arctan domain is [-pi/2, pi/2])
    score_d = min(((f_d - mu_d) / (sqrt2 sigma_d))^2, ln(q_d/(EPS sqrt(pi))))
    e = s_w * sum_d score_d
All signs are folded into squared terms; normalisations go through exp/ln
(ScalarE Rsqrt/Reciprocal are disallowed).

Layout per core/chunk: partition p = (batch, chain, block); free dim =
plane-contiguous arrays of R residue pairs (unit-stride runs for the DVE).
Two chunks pipeline DMA/DVE/ACT/GPSIMD; all Ln/Exp activations are emitted
for both chunks before any Arctan so the ACT table set loads exactly twice.
"""

import os
import numpy as np

PAD = -999.0
PAD_I = -999
NB, MC, MR = 64, 4, 2048
NALT = 2
NCORES = 8
BPC = NB // NCORES            # batches per core
CH = int(os.environ.get("BLC_CHUNKS", "2"))  # pipeline chunks per core
KC = (4 * CH) // 1            # blocks per (batch, chain) across full chain
R = MR // KC                  # residues (pairs) per partition
EPS = 1e-12
CL = 1.0 / (EPS * np.sqrt(np.pi))

_PROGRAM_CACHE = {}
LAST_RESULT = None            # BassKernelResults of the last run (for test.py)
TRACE = bool(int(os.environ.get("BLC_TRACE", "0")))


def _build_program():
    import concourse.bass as bass
    import concourse.tile as tile
    from concourse import bacc, mybir

    dt = mybir.dt.float32
    Alu = mybir.AluOpType
    Act = mybir.ActivationFunctionType

    from concourse.bass import _add_dep_helper

    nc = bacc.Bacc("TRN2", target_bir_lowering=False, debug=False)
    G_t = nc.declare_dram_parameter("g", [BPC, MC, KC, 9, R + 1], dt,
                                    isOutput=False)
    P_t = nc.declare_dram_parameter("pr", [BPC, MC, KC, 9, R], dt,
                                    isOutput=False)
    O_t = nc.declare_dram_parameter("out", [BPC, MC, MR], dt, isOutput=True)

    bc = BPC // CH            # batches per chunk
    bufs = min(CH, 2)
    S = R + 1                 # slots per coord plane

    with tile.TileContext(nc) as tc:
        with (
            tc.tile_pool(name="px", bufs=bufs) as px,
            tc.tile_pool(name="pp", bufs=bufs) as pp,
            tc.tile_pool(name="ps", bufs=bufs) as ps,
        ):
            state = []
            # ---------------- phase 1: loads, geometry, ln/exp -------------
            for c in range(CH):
                b0 = c * bc
                # one DMA per batch -> spreads across HWDGE queues
                X = px.tile([128, 9 * S], dt, tag="x")
                P = pp.tile([128, 9 * R], dt, tag="p")
                pb = 128 // bc
                for i in range(bc):
                    nc.sync.dma_start(X[pb * i:pb * (i + 1), :], G_t[b0 + i])
                    nc.sync.dma_start(P[pb * i:pb * (i + 1), :], P_t[b0 + i])

                def xpl(p, off):   # coord plane p, slot offset
                    return X[:, p * S + off: p * S + off + R]

                # difference vectors, plane-contiguous [v2|v1|v3] x (x,y,z)
                V = px.tile([128, 9 * R], dt, tag="v")
                Vv = V[:].rearrange("p (v c l) -> p v c l", v=3, c=3)
                Xv = X[:].rearrange("p (n l) -> p n l", n=9)
                # v2 = CA(l+1) - N(l+1)
                nc.vector.tensor_sub(Vv[:, 0], Xv[:, 3:6, 1:S], Xv[:, 0:3, 1:S])
                # v1 = C(l) - N(l+1)
                nc.vector.tensor_sub(Vv[:, 1], Xv[:, 6:9, 0:R], Xv[:, 0:3, 1:S])
                # v3 = CA(l) - C(l)
                nc.vector.tensor_sub(Vv[:, 2], Xv[:, 3:6, 0:R], Xv[:, 6:9, 0:R])

                SQ = px.tile([128, 9 * R], dt, tag="sq")
                nc.scalar.activation(SQ[:], V[:], Act.Square)
                SQv = SQ[:].rearrange("p (v c l) -> p v c l", v=3, c=3)
                # D1 = [d22 | d11 | d33]
                D1 = ps.tile([128, 3 * R], dt, tag="d1")
                D1v = D1[:].rearrange("p (v l) -> p v l", v=3)
                nc.vector.tensor_add(D1v, SQv[:, :, 0], SQv[:, :, 1])
                nc.vector.tensor_add(D1v, D1v, SQv[:, :, 2])
                # cross products [v1*v2 | v3*v1]
                CP = ps.tile([128, 6 * R], dt, tag="cp")
                nc.vector.tensor_mul(CP[:], V[:, 3 * R:9 * R], V[:, 0:6 * R])
                CPv = CP[:].rearrange("p (m c l) -> p m c l", m=2, c=3)
                DC = ps.tile([128, 2 * R], dt, tag="dc")
                DCv = DC[:].rearrange("p (m l) -> p m l", m=2)
                nc.vector.tensor_add(DCv, CPv[:, :, 0], CPv[:, :, 1])
                nc.vector.tensor_add(DCv, DCv, CPv[:, :, 2])

                M = ps.tile([128, 2 * R], dt, tag="m")
                nc.vector.tensor_mul(M[:, 0:R], D1[:, 0:R], D1[:, R:2 * R])
                nc.vector.tensor_mul(M[:, R:2 * R], D1[:, 2 * R:3 * R],
                                     D1[:, R:2 * R])
                PSQ = ps.tile([128, 2 * R], dt, tag="psq")
                nc.scalar.activation(PSQ[:], DC[:], Act.Square)
                # sg = sign(c + eps) in {-1, +1}, on GPSIMD (keeps the ACT
                # engine's function set stable: only Square/Ln/Exp/Arctan)
                SG = ps.tile([128, 2 * R], dt, tag="sg")
                nc.gpsimd.tensor_scalar(SG[:], DC[:], -1e-35, None,
                                        op0=Alu.is_ge)
                nc.gpsimd.tensor_scalar(SG[:], SG[:], 2.0, -1.0,
                                        op0=Alu.mult, op1=Alu.add)
                S2 = ps.tile([128, 2 * R], dt, tag="s2")
                nc.vector.tensor_sub(S2[:], M[:], PSQ[:])
                LNIN = ps.tile([128, 5 * R], dt, tag="lnin")
                nc.vector.tensor_scalar_max(LNIN[:, 0:2 * R], S2[:], 1e-30)
                nc.vector.tensor_mul(LNIN[:, 2 * R:4 * R], DC[:], SG[:])
                nc.vector.tensor_scalar_max(
                    LNIN[:, 2 * R:4 * R], LNIN[:, 2 * R:4 * R], 1e-35)
                nc.vector.tensor_scalar_max(LNIN[:, 4 * R:5 * R],
                                            D1[:, R:2 * R], 1e-30)
                LNO = ps.tile([128, 5 * R], dt, tag="lno")
                nc.scalar.activation(LNO[:], LNIN[:], Act.Ln)
                RT = ps.tile([128, 2 * R], dt, tag="rt")
                nc.vector.scalar_tensor_tensor(
                    RT[:], LNO[:, 0:2 * R], -0.5, LNO[:, 2 * R:4 * R],
                    op0=Alu.mult, op1=Alu.add)
                SR = ps.tile([128, 2 * R], dt, tag="sr")
                nc.gpsimd.tensor_scalar(SR[:], RT[:], 0.0, None, op0=Alu.is_ge)
                nc.gpsimd.tensor_scalar(SR[:], SR[:], 2.0, -1.0,
                                        op0=Alu.mult, op1=Alu.add)
                ABSR = ps.tile([128, 2 * R], dt, tag="absr")
                nc.vector.tensor_mul(ABSR[:], RT[:], SR[:])
                EN = ps.tile([128, 2 * R], dt, tag="en")
                nc.scalar.activation(EN[:], ABSR[:], Act.Exp, scale=-1.0)
                F1 = ps.tile([128, R], dt, tag="f1")
                f1_inst = nc.scalar.activation(F1[:], LNO[:, 4 * R:5 * R],
                                               Act.Exp, scale=0.5)
                state.append((b0, P, EN, SR, SG, F1, f1_inst))

            # ---------------- phase 2: arctan + scoring --------------------
            for c in range(CH):
                b0, P, EN, SR, SG, F1, _ = state[c]
                TP = ps.tile([128, 2 * R], dt, tag="tp")
                tp_inst = nc.scalar.activation(TP[:], EN[:], Act.Arctan)
                if c == 0:
                    # keep the ACT engine in the ln/exp table set until every
                    # chunk's ln/exp work is done (arctan lives in another set)
                    for (_, _, _, _, _, _, fi) in state[1:]:
                        _add_dep_helper(
                            tp_inst.ins, fi.ins,
                            reason="ACT table-set ordering: arctan after all exp")
                TB = ps.tile([128, 2 * R], dt, tag="tb")
                nc.vector.tensor_scalar(TB[:], TP[:], -1.0, np.pi / 4,
                                        op0=Alu.mult, op1=Alu.add)
                TC = ps.tile([128, 2 * R], dt, tag="tc")
                nc.vector.tensor_mul(TC[:], SR[:], TB[:])
                AV = ps.tile([128, 2 * R], dt, tag="av")
                nc.vector.tensor_mul(AV[:], SG[:], P[:, R:3 * R])
                W = ps.tile([128, 3 * R], dt, tag="w")
                nc.vector.tensor_mul(W[:, 0:R], F1[:], P[:, 3 * R:4 * R])
                nc.vector.scalar_tensor_tensor(
                    W[:, R:3 * R], TC[:], np.pi / 4, P[:, 4 * R:6 * R],
                    op0=Alu.add, op1=Alu.mult)
                U = ps.tile([128, 3 * R], dt, tag="u")
                nc.vector.tensor_sub(U[:, 0:R], W[:, 0:R], P[:, 0:R])
                nc.vector.tensor_sub(U[:, R:3 * R], W[:, R:3 * R], AV[:])
                Z = ps.tile([128, 3 * R], dt, tag="z")
                nc.gpsimd.tensor_mul(Z[:], U[:], U[:])
                ZC = ps.tile([128, 3 * R], dt, tag="zc")
                nc.vector.tensor_tensor(ZC[:], Z[:], P[:, 6 * R:9 * R],
                                        op=Alu.min)
                E = ps.tile([128, R], dt, tag="e")
                nc.gpsimd.tensor_add(E[:], ZC[:, 0:R], ZC[:, R:2 * R])
                nc.gpsimd.tensor_add(E[:], E[:], ZC[:, 2 * R:3 * R])
                nc.sync.dma_start(
                    O_t[b0:b0 + bc].rearrange("b c (k l) -> b c k l", k=KC),
                    E[:])
    return nc


def _get_program():
    if "nc" not in _PROGRAM_CACHE:
        nc = _build_program()
        nc.finalize()   # Bacc: register allocation / DCE / wait legalization
        _PROGRAM_CACHE["nc"] = nc
    return _PROGRAM_CACHE["nc"]


def _host_prep(atom_description, coords, mean, std, weight):
    ad = np.asarray(atom_description)
    coords = np.asarray(coords, dtype=np.float32)
    b, ch, rs, rn, an = (ad[:, i] for i in range(5))
    valid = (b >= 0) & (b < NB) & (ch >= 0) & (ch < MC) & (rs >= 0) & (rs < MR)

    def scat3(mask):
        A = np.full((NB, MC, MR, 3), PAD, np.float32)
        m = mask & valid
        A[b[m], ch[m], rs[m]] = coords[m]
        return A

    Narr, CAarr, Carr = scat3(an == 0), scat3(an == 1), scat3(an == 2)
    seq = np.full((NB, MC, MR), PAD_I, np.int64)
    m = (an == 1) & valid
    seq[b[m], ch[m], rs[m]] = rn[m]

    todo = ((Narr[:, :, 1:, 0] != PAD) & (Carr[:, :, :-1, 0] != PAD)
            & (CAarr[:, :, 1:, 0] != PAD) & (CAarr[:, :, :-1, 0] != PAD)
            & (seq[:, :, 1:] != PAD_I) & (seq[:, :, :-1] != PAD_I))
    sidx = np.clip(np.where(todo, seq[:, :, 1:], 0), 0, 19)

    w0 = float(np.asarray(weight).reshape(-1)[0])
    s_w = 1.0 - np.tanh(-w0)
    sq = np.sqrt(s_w)
    mu = np.asarray(mean, np.float64)
    sd = np.asarray(std, np.float64)
    q = 1.0 / (sd * np.sqrt(2.0))
    tab = np.empty((20, 9))
    tab[:, 0] = mu[:, 0] * q[:, 0] * sq
    tab[:, 1] = (np.pi / 2 - mu[:, 1]) * q[:, 1] * sq
    tab[:, 2] = (mu[:, 2] - np.pi / 2) * q[:, 2] * sq
    tab[:, 3:6] = q * sq
    tab[:, 6:9] = s_w * np.maximum(np.log(CL * q), 0.0)
    tab = tab.astype(np.float32)

    params = np.zeros((NB, MC, MR, 9), np.float32)
    params[:, :, 1:, :] = tab[sidx] * todo[..., None].astype(np.float32)
    # blocked coefficient-plane layout [NB, MC, KC, 9, R]
    pblk = np.ascontiguousarray(
        params.reshape(NB, MC, KC, R, 9).transpose(0, 1, 2, 4, 3))

    G = np.zeros((NB, MC, MR + 1, 9), np.float32)
    G[:, :, 1:, 0:3] = Narr
    G[:, :, 1:, 3:6] = CAarr
    G[:, :, 1:, 6:9] = Carr
    # blocked plane-contiguous with halo: GB[b,c,k,p,l] = G[b,c,k*R+l,p]
    GB = np.empty((NB, MC, KC, 9, R + 1), np.float32)
    for k in range(KC):
        GB[:, :, k] = G[:, :, k * R:k * R + R + 1, :].transpose(0, 1, 3, 2)
    return GB, pblk


def _install_ntff_hook():
    """The agent image's antenv lacks axon_hooks; synthesize it so
    trace=True can reach the terminal's NRT profiler (dev-only path)."""
    import sys, types
    if "antenv.axon_hooks" in sys.modules:
        return True
    try:
        import antenv
        mod = types.ModuleType("antenv.axon_hooks")
        mod._hook = None

        def set_axon_ntff_profile_hook(h):
            mod._hook = h

        def get_axon_ntff_profile_hook():
            return mod._hook

        mod.set_axon_ntff_profile_hook = set_axon_ntff_profile_hook
        mod.get_axon_ntff_profile_hook = get_axon_ntff_profile_hook
        sys.modules["antenv.axon_hooks"] = mod
        antenv.axon_hooks = mod
        from trn_agent_boot.trn_boot import _ntff_profile_via_ctypes
        mod._hook = _ntff_profile_via_ctypes("/opt/axon/libaxon_pjrt.so")
        return True
    except Exception as e:  # pragma: no cover - profiling is best-effort
        print(f"ntff hook install failed: {e}")
        return False


def kernel(**inputs):
    global LAST_RESULT
    from concourse.bass_utils import run_bass_kernel_spmd
    if TRACE:
        _install_ntff_hook()

    G, pblk = _host_prep(
        inputs["atom_description"], inputs["coords"],
        inputs["mean"], inputs["std"], inputs["weight"])

    nc = _get_program()
    in_maps = [
        {"g": np.ascontiguousarray(G[i * BPC:(i + 1) * BPC]),
         "pr": np.ascontiguousarray(pblk[i * BPC:(i + 1) * BPC])}
        for i in range(NCORES)
    ]
    res = run_bass_kernel_spmd(nc, in_maps, list(range(NCORES)), trace=TRACE)
    LAST_RESULT = res
    e = np.concatenate([res.results[i]["out"] for i in range(NCORES)], axis=0)
    e = e.reshape(NB, MC, MR)
    out = np.repeat(e[..., None], NALT, axis=-1)
    return np.ascontiguousarray(out.astype(np.float32))


# revision 29
# speedup vs baseline: 1.3408x; 1.3408x over previous
"""Trainium2 Bass kernel for nn_BondLenConstrain.

Contract: kernel(**inputs) takes the FULL (unsharded) inputs of
reference.setup_inputs() and returns the full [64, 4, 2048, 2] float32
resiEnergy tensor.  Data-parallel over the batch axis across 8 NeuronCores
(8 batches per core).

Host (numpy, indexing only): scatter atoms into dense residue grids exactly
like the reference, build the `todo` mask, gather the tiny per-residue-type
tables into per-residue coefficient planes (masked pairs get all-zero
coefficients -> device formula returns exactly 0), transpose coords to a
plane-contiguous blocked layout, and broadcast the (identical) nalt lanes
of the output on assembly.

Device math per residue pair (r-1, r):
    v2 = CA_r - N_r, v1 = C_{r-1} - N_r, v3 = CA_{r-1} - C_{r-1}
    bond  f1 = sqrt(d11) = exp(0.5 ln d11)
    angle theta = pi/2 - sign(c) * arctan(|c|/s),  s = sqrt(dxx*d11 - c^2)
          arctan over [0,inf) via  t' = arctan(exp(-|ln(|c|/s)|)) in [0,pi/4]
          (ScalarE arctan domain is [-pi/2, pi/2])
    score_d = min(((f_d - mu_d) / (sqrt2 sigma_d))^2, ln(q_d/(EPS sqrt(pi))))
    e = s_w * sum_d score_d
Signs are folded into squared terms; normalisations go through exp/ln
(ScalarE Rsqrt/Reciprocal are disallowed).

Scheduling notes:
  * plane-contiguous free-dim layouts -> every DVE op streams unit-stride.
  * two chunks pipeline DMA/DVE/ACT/GPSIMD; per-batch DMAs spread queues.
  * walrus reloads the ACT function table on every Ln/Exp/Arctan function
    CHANGE (Square/Sign are fillers present in every set), so Ln / Exp /
    Arctan are emitted grouped across chunks and ordered with explicit
    deps: 3 table loads total, the first hidden in the DMA fill by a dummy.
"""

import os
import numpy as np

PAD = -999.0
PAD_I = -999
NB, MC, MR = 64, 4, 2048
NALT = 2
NCORES = 8
BPC = NB // NCORES            # batches per core
CH = int(os.environ.get("BLC_CHUNKS", "2"))  # pipeline chunks per core
KC = 4 * CH                   # blocks per (batch, chain) across full chain
R = MR // KC                  # residues (pairs) per partition
EPS = 1e-12
CL = 1.0 / (EPS * np.sqrt(np.pi))

_PROGRAM_CACHE = {}
LAST_RESULT = None            # BassKernelResults of the last run (for test.py)
TRACE = bool(int(os.environ.get("BLC_TRACE", "0")))


def _build_program():
    import concourse.bass as bass
    import concourse.tile as tile
    from concourse import bacc, mybir
    from concourse.bass import _add_dep_helper

    dt = mybir.dt.float32
    Alu = mybir.AluOpType
    Act = mybir.ActivationFunctionType

    nc = bacc.Bacc("TRN2", target_bir_lowering=False, debug=False)
    # const AP for the Sign bias (maps c == 0 to sign +1)
    _sgn_eps = 1e-35
    _ct = nc.alloc_sbuf_tensor("const-f32-sgneps", [128, 1], dt)
    nc.gpsimd.memset(_ct.ap(), _sgn_eps)
    nc.const_aps.aps[(dt, _sgn_eps)] = _ct.ap()
    nc.all_engine_barrier()

    G_t = nc.declare_dram_parameter("g", [BPC, MC, KC, 9, R + 1], dt,
                                    isOutput=False)
    P_t = nc.declare_dram_parameter("pr", [BPC, MC, KC, 9, R], dt,
                                    isOutput=False)
    O_t = nc.declare_dram_parameter("out", [BPC, MC, MR], dt, isOutput=True)

    bc = BPC // CH            # batches per chunk
    bufs = min(CH, 2)
    S = R + 1                 # slots per coord plane

    with tile.TileContext(nc) as tc:
        with (
            tc.tile_pool(name="px", bufs=bufs) as px,
            tc.tile_pool(name="pp", bufs=bufs) as pp,
            tc.tile_pool(name="ps", bufs=bufs) as ps,
        ):
            # dummy Ln: pulls the natural_log table load into the DMA fill
            dscr = ps.tile([128, 1], dt, tag="dummy")
            nc.scalar.activation(dscr[:], _ct.ap(), Act.Ln)

            st1, st2, st3 = [], [], []
            # -------- phase 1: loads, geometry, up to the Ln input ---------
            for c in range(CH):
                b0 = c * bc
                X = px.tile([128, 9 * S], dt, tag="x")
                P = pp.tile([128, 9 * R], dt, tag="p")
                pb = 128 // bc
                for i in range(bc):       # per-batch DMAs -> parallel queues
                    nc.sync.dma_start(X[pb * i:pb * (i + 1), :], G_t[b0 + i])
                    nc.sync.dma_start(P[pb * i:pb * (i + 1), :], P_t[b0 + i])

                # difference vectors, plane-contiguous [v2|v1|v3] x (x,y,z)
                V = px.tile([128, 9 * R], dt, tag="v")
                Vv = V[:].rearrange("p (v c l) -> p v c l", v=3, c=3)
                Xv = X[:].rearrange("p (n l) -> p n l", n=9)
                nc.vector.tensor_sub(Vv[:, 0], Xv[:, 3:6, 1:S], Xv[:, 0:3, 1:S])
                nc.vector.tensor_sub(Vv[:, 1], Xv[:, 6:9, 0:R], Xv[:, 0:3, 1:S])
                nc.vector.tensor_sub(Vv[:, 2], Xv[:, 3:6, 0:R], Xv[:, 6:9, 0:R])

                SQ = px.tile([128, 9 * R], dt, tag="sq")
                nc.scalar.activation(SQ[:], V[:], Act.Square)
                SQv = SQ[:].rearrange("p (v c l) -> p v c l", v=3, c=3)
                D1 = ps.tile([128, 3 * R], dt, tag="d1")   # [d22|d11|d33]
                D1v = D1[:].rearrange("p (v l) -> p v l", v=3)
                nc.vector.tensor_add(D1v, SQv[:, :, 0], SQv[:, :, 1])
                nc.vector.tensor_add(D1v, D1v, SQv[:, :, 2])
                CP = ps.tile([128, 6 * R], dt, tag="cp")   # [v1*v2|v3*v1]
                nc.gpsimd.tensor_mul(CP[:], V[:, 3 * R:9 * R], V[:, 0:6 * R])
                CPv = CP[:].rearrange("p (m c l) -> p m c l", m=2, c=3)
                DC = ps.tile([128, 2 * R], dt, tag="dc")   # [d12|d31]
                DCv = DC[:].rearrange("p (m l) -> p m l", m=2)
                nc.vector.tensor_add(DCv, CPv[:, :, 0], CPv[:, :, 1])
                nc.vector.tensor_add(DCv, DCv, CPv[:, :, 2])

                M = ps.tile([128, 2 * R], dt, tag="m")
                nc.vector.tensor_mul(M[:, 0:R], D1[:, 0:R], D1[:, R:2 * R])
                nc.vector.tensor_mul(M[:, R:2 * R], D1[:, 2 * R:3 * R],
                                     D1[:, R:2 * R])
                PSQ = ps.tile([128, 2 * R], dt, tag="psq")
                nc.scalar.activation(PSQ[:], DC[:], Act.Square)
                SG = ps.tile([128, 2 * R], dt, tag="sg")
                nc.scalar.activation(SG[:], DC[:], Act.Sign, bias=1e-35)
                S2 = ps.tile([128, 2 * R], dt, tag="s2")
                nc.vector.tensor_sub(S2[:], M[:], PSQ[:])
                LNIN = ps.tile([128, 5 * R], dt, tag="lnin")
                nc.vector.tensor_scalar_max(LNIN[:, 0:2 * R], S2[:], 1e-30)
                nc.vector.tensor_mul(LNIN[:, 2 * R:4 * R], DC[:], SG[:])
                nc.vector.tensor_scalar_max(
                    LNIN[:, 2 * R:4 * R], LNIN[:, 2 * R:4 * R], 1e-35)
                nc.vector.tensor_scalar_max(LNIN[:, 4 * R:5 * R],
                                            D1[:, R:2 * R], 1e-30)
                st1.append((b0, P, SG, LNIN))

            # -------- phase 2: all Ln -------------------------------------
            for c in range(CH):
                b0, P, SG, LNIN = st1[c]
                LNO = ps.tile([128, 5 * R], dt, tag="lno")
                ln_i = nc.scalar.activation(LNO[:], LNIN[:], Act.Ln)
                st2.append((b0, P, SG, LNO, ln_i))

            # -------- phase 3: r/sign/|r| then all Exp --------------------
            st3 = []
            for c in range(CH):
                b0, P, SG, LNO, _ = st2[c]
                RT = ps.tile([128, 2 * R], dt, tag="rt")
                nc.vector.scalar_tensor_tensor(
                    RT[:], LNO[:, 0:2 * R], -0.5, LNO[:, 2 * R:4 * R],
                    op0=Alu.mult, op1=Alu.add)
                SR = ps.tile([128, 2 * R], dt, tag="sr")
                nc.scalar.activation(SR[:], RT[:], Act.Sign)
                ABSR = ps.tile([128, 2 * R], dt, tag="absr")
                nc.vector.tensor_mul(ABSR[:], RT[:], SR[:])
                EN = ps.tile([128, 2 * R], dt, tag="en")
                en_i = nc.scalar.activation(EN[:], ABSR[:], Act.Exp, scale=-1.0)
                F1 = ps.tile([128, R], dt, tag="f1")
                f1_i = nc.scalar.activation(F1[:], LNO[:, 4 * R:5 * R],
                                            Act.Exp, scale=0.5)
                if c == 0:   # no Exp before the last chunk's Ln
                    _add_dep_helper(en_i.ins, st2[-1][4].ins,
                                    reason="group ACT Ln before Exp")
                st3.append((b0, P, SG, SR, EN, F1, f1_i))

            # -------- phase 4: all Arctan + scoring -----------------------
            for c in range(CH):
                b0, P, SG, SR, EN, F1, _ = st3[c]
                TP = ps.tile([128, 2 * R], dt, tag="tp")
                tp_i = nc.scalar.activation(TP[:], EN[:], Act.Arctan)
                if c == 0:   # no Arctan before the last chunk's Exp
                    _add_dep_helper(tp_i.ins, st3[-1][6].ins,
                                    reason="group ACT Exp before Arctan")
                TB = ps.tile([128, 2 * R], dt, tag="tb")
                nc.vector.tensor_scalar(TB[:], TP[:], -1.0, np.pi / 4,
                                        op0=Alu.mult, op1=Alu.add)
                TC = ps.tile([128, 2 * R], dt, tag="tc")
                nc.vector.tensor_mul(TC[:], SR[:], TB[:])
                AV = ps.tile([128, 2 * R], dt, tag="av")
                nc.vector.tensor_mul(AV[:], SG[:], P[:, R:3 * R])
                W = ps.tile([128, 3 * R], dt, tag="w")
                nc.vector.tensor_mul(W[:, 0:R], F1[:], P[:, 3 * R:4 * R])
                nc.vector.scalar_tensor_tensor(
                    W[:, R:3 * R], TC[:], np.pi / 4, P[:, 4 * R:6 * R],
                    op0=Alu.add, op1=Alu.mult)
                U = ps.tile([128, 3 * R], dt, tag="u")
                nc.vector.tensor_sub(U[:, 0:R], W[:, 0:R], P[:, 0:R])
                nc.vector.tensor_sub(U[:, R:3 * R], W[:, R:3 * R], AV[:])
                Z = ps.tile([128, 3 * R], dt, tag="z")
                nc.gpsimd.tensor_mul(Z[:], U[:], U[:])
                ZC = ps.tile([128, 3 * R], dt, tag="zc")
                nc.vector.tensor_tensor(ZC[:], Z[:], P[:, 6 * R:9 * R],
                                        op=Alu.min)
                E = ps.tile([128, R], dt, tag="e")
                nc.gpsimd.tensor_add(E[:], ZC[:, 0:R], ZC[:, R:2 * R])
                nc.gpsimd.tensor_add(E[:], E[:], ZC[:, 2 * R:3 * R])
                nc.sync.dma_start(
                    O_t[b0:b0 + bc].rearrange("b c (k l) -> b c k l", k=KC),
                    E[:])
    return nc


def _get_program():
    if "nc" not in _PROGRAM_CACHE:
        nc = _build_program()
        nc.finalize()   # Bacc: register allocation / DCE / wait legalization
        _PROGRAM_CACHE["nc"] = nc
    return _PROGRAM_CACHE["nc"]


def _host_prep(atom_description, coords, mean, std, weight):
    ad = np.asarray(atom_description)
    coords = np.asarray(coords, dtype=np.float32)
    b, ch, rs, rn, an = (ad[:, i] for i in range(5))
    valid = (b >= 0) & (b < NB) & (ch >= 0) & (ch < MC) & (rs >= 0) & (rs < MR)

    def scat3(mask):
        A = np.full((NB, MC, MR, 3), PAD, np.float32)
        m = mask & valid
        A[b[m], ch[m], rs[m]] = coords[m]
        return A

    Narr, CAarr, Carr = scat3(an == 0), scat3(an == 1), scat3(an == 2)
    seq = np.full((NB, MC, MR), PAD_I, np.int64)
    m = (an == 1) & valid
    seq[b[m], ch[m], rs[m]] = rn[m]

    todo = ((Narr[:, :, 1:, 0] != PAD) & (Carr[:, :, :-1, 0] != PAD)
            & (CAarr[:, :, 1:, 0] != PAD) & (CAarr[:, :, :-1, 0] != PAD)
            & (seq[:, :, 1:] != PAD_I) & (seq[:, :, :-1] != PAD_I))
    sidx = np.clip(np.where(todo, seq[:, :, 1:], 0), 0, 19)

    w0 = float(np.asarray(weight).reshape(-1)[0])
    s_w = 1.0 - np.tanh(-w0)
    sq = np.sqrt(s_w)
    mu = np.asarray(mean, np.float64)
    sd = np.asarray(std, np.float64)
    q = 1.0 / (sd * np.sqrt(2.0))
    tab = np.empty((20, 9))
    tab[:, 0] = mu[:, 0] * q[:, 0] * sq
    tab[:, 1] = (np.pi / 2 - mu[:, 1]) * q[:, 1] * sq
    tab[:, 2] = (mu[:, 2] - np.pi / 2) * q[:, 2] * sq
    tab[:, 3:6] = q * sq
    tab[:, 6:9] = s_w * np.maximum(np.log(CL * q), 0.0)
    tab = tab.astype(np.float32)

    params = np.zeros((NB, MC, MR, 9), np.float32)
    params[:, :, 1:, :] = tab[sidx] * todo[..., None].astype(np.float32)
    # blocked coefficient-plane layout [NB, MC, KC, 9, R]
    pblk = np.ascontiguousarray(
        params.reshape(NB, MC, KC, R, 9).transpose(0, 1, 2, 4, 3))

    G = np.zeros((NB, MC, MR + 1, 9), np.float32)
    G[:, :, 1:, 0:3] = Narr
    G[:, :, 1:, 3:6] = CAarr
    G[:, :, 1:, 6:9] = Carr
    # blocked plane-contiguous with halo: GB[b,c,k,p,l] = G[b,c,k*R+l,p]
    GB = np.empty((NB, MC, KC, 9, R + 1), np.float32)
    for k in range(KC):
        GB[:, :, k] = G[:, :, k * R:k * R + R + 1, :].transpose(0, 1, 3, 2)
    return GB, pblk


def _install_ntff_hook():
    """The agent image's antenv lacks axon_hooks; synthesize it so
    trace=True can reach the terminal's NRT profiler (dev-only path)."""
    import sys, types
    if "antenv.axon_hooks" in sys.modules:
        return True
    try:
        import antenv
        mod = types.ModuleType("antenv.axon_hooks")
        mod._hook = None

        def set_axon_ntff_profile_hook(h):
            mod._hook = h

        def get_axon_ntff_profile_hook():
            return mod._hook

        mod.set_axon_ntff_profile_hook = set_axon_ntff_profile_hook
        mod.get_axon_ntff_profile_hook = get_axon_ntff_profile_hook
        sys.modules["antenv.axon_hooks"] = mod
        antenv.axon_hooks = mod
        from trn_agent_boot.trn_boot import _ntff_profile_via_ctypes
        mod._hook = _ntff_profile_via_ctypes("/opt/axon/libaxon_pjrt.so")
        return True
    except Exception as e:  # pragma: no cover - profiling is best-effort
        print(f"ntff hook install failed: {e}")
        return False


def kernel(**inputs):
    global LAST_RESULT
    from concourse.bass_utils import run_bass_kernel_spmd
    if TRACE:
        _install_ntff_hook()

    G, pblk = _host_prep(
        inputs["atom_description"], inputs["coords"],
        inputs["mean"], inputs["std"], inputs["weight"])

    nc = _get_program()
    in_maps = [
        {"g": np.ascontiguousarray(G[i * BPC:(i + 1) * BPC]),
         "pr": np.ascontiguousarray(pblk[i * BPC:(i + 1) * BPC])}
        for i in range(NCORES)
    ]
    res = run_bass_kernel_spmd(nc, in_maps, list(range(NCORES)), trace=TRACE)
    LAST_RESULT = res
    e = np.concatenate([res.results[i]["out"] for i in range(NCORES)], axis=0)
    e = e.reshape(NB, MC, MR)
    out = np.repeat(e[..., None], NALT, axis=-1)
    return np.ascontiguousarray(out.astype(np.float32))


# revision 33
# speedup vs baseline: 1.4723x; 1.0981x over previous
"""Trainium2 Bass kernel for nn_BondLenConstrain.

Contract: kernel(**inputs) takes the FULL (unsharded) inputs of
reference.setup_inputs() and returns the full [64, 4, 2048, 2] float32
resiEnergy tensor.  Data-parallel over the batch axis across 8 NeuronCores
(8 batches per core).

Host (numpy, indexing only): scatter atoms into dense residue grids exactly
like the reference, build the `todo` mask, gather the tiny per-residue-type
tables into per-residue coefficient planes (masked pairs get all-zero
coefficients -> device formula returns exactly 0), transpose coords to a
plane-contiguous blocked layout, and broadcast the (identical) nalt lanes
of the output on assembly.

Device math per residue pair (r-1, r):
    v2 = CA_r - N_r, v1 = C_{r-1} - N_r, v3 = CA_{r-1} - C_{r-1}
    bond  f1 = sqrt(d11) = exp(0.5 ln d11)
    angle theta = pi/2 - sign(c) * arctan(|c|/s),  s = sqrt(dxx*d11 - c^2)
          arctan over [0,inf) via  t' = arctan(exp(-|ln(|c|/s)|)) in [0,pi/4]
          (ScalarE arctan domain is [-pi/2, pi/2])
    score_d = min(((f_d - mu_d) / (sqrt2 sigma_d))^2, ln(q_d/(EPS sqrt(pi))))
    e = s_w * sum_d score_d
Signs are folded into squared terms; normalisations go through exp/ln
(ScalarE Rsqrt/Reciprocal are disallowed).

Scheduling notes:
  * plane-contiguous free-dim layouts -> every DVE op streams unit-stride.
  * two chunks pipeline DMA/DVE/ACT/GPSIMD; per-batch DMAs spread queues.
  * walrus reloads the ACT function table on every Ln/Exp/Arctan function
    CHANGE (Square/Sign are fillers present in every set), so Ln / Exp /
    Arctan are emitted grouped across chunks and ordered with explicit
    deps: 3 table loads total, the first hidden in the DMA fill by a dummy.
"""

import os
import numpy as np

PAD = -999.0
PAD_I = -999
NB, MC, MR = 64, 4, 2048
NALT = 2
NCORES = 8
BPC = NB // NCORES            # batches per core
CH = int(os.environ.get("BLC_CHUNKS", "2"))  # pipeline chunks per core
KC = 4 * CH                   # blocks per (batch, chain) across full chain
R = MR // KC                  # residues (pairs) per partition
EPS = 1e-12
CL = 1.0 / (EPS * np.sqrt(np.pi))

_PROGRAM_CACHE = {}
LAST_RESULT = None            # BassKernelResults of the last run (for test.py)
TRACE = bool(int(os.environ.get("BLC_TRACE", "0")))


def _build_program():
    import concourse.bass as bass
    import concourse.tile as tile
    from concourse import bacc, mybir
    from concourse.bass import _add_dep_helper

    dt = mybir.dt.float32
    Alu = mybir.AluOpType
    Act = mybir.ActivationFunctionType

    nc = bacc.Bacc("TRN2", target_bir_lowering=False, debug=False)
    # const AP for the Sign bias (maps c == 0 to sign +1)
    _sgn_eps = 1e-35
    _ct = nc.alloc_sbuf_tensor("const-f32-sgneps", [128, 1], dt)
    nc.gpsimd.memset(_ct.ap(), _sgn_eps)
    nc.const_aps.aps[(dt, _sgn_eps)] = _ct.ap()
    nc.all_engine_barrier()

    G_t = nc.declare_dram_parameter("g", [BPC, MC, KC, 9, R + 1], dt,
                                    isOutput=False)
    P_t = nc.declare_dram_parameter("pr", [BPC, MC, KC, 9, R], dt,
                                    isOutput=False)
    O_t = nc.declare_dram_parameter("out", [BPC, MC, MR], dt, isOutput=True)

    bc = BPC // CH            # batches per chunk
    bufs = min(CH, 2)
    S = R + 1                 # slots per coord plane

    with tile.TileContext(nc) as tc:
        with (
            tc.tile_pool(name="px", bufs=bufs) as px,
            tc.tile_pool(name="pp", bufs=bufs) as pp,
            tc.tile_pool(name="ps", bufs=bufs) as ps,
        ):
            # dummy Ln: pulls the natural_log table load into the DMA fill
            dscr = ps.tile([128, 1], dt, tag="dummy")
            nc.scalar.activation(dscr[:], _ct.ap(), Act.Ln)

            st1, st2, st3 = [], [], []
            # -------- phase 1: loads, geometry, up to the Ln input ---------
            for c in range(CH):
                b0 = c * bc
                X = px.tile([128, 9 * S], dt, tag="x")
                P = pp.tile([128, 9 * R], dt, tag="p")
                # X on the sync HWDGE ring, P on the (early-idle) ACT ring:
                # both issue + transfer in parallel
                nc.sync.dma_start(X[:], G_t[b0:b0 + bc])
                nc.scalar.dma_start(P[:], P_t[b0:b0 + bc])

                # difference vectors, plane-contiguous [v2|v1|v3] x (x,y,z)
                V = px.tile([128, 9 * R], dt, tag="v")
                Vv = V[:].rearrange("p (v c l) -> p v c l", v=3, c=3)
                Xv = X[:].rearrange("p (n l) -> p n l", n=9)
                nc.vector.tensor_sub(Vv[:, 0], Xv[:, 3:6, 1:S], Xv[:, 0:3, 1:S])
                nc.vector.tensor_sub(Vv[:, 1], Xv[:, 6:9, 0:R], Xv[:, 0:3, 1:S])
                nc.vector.tensor_sub(Vv[:, 2], Xv[:, 3:6, 0:R], Xv[:, 6:9, 0:R])

                SQ = px.tile([128, 9 * R], dt, tag="sq")
                nc.scalar.activation(SQ[:], V[:], Act.Square)
                SQv = SQ[:].rearrange("p (v c l) -> p v c l", v=3, c=3)
                D1 = ps.tile([128, 3 * R], dt, tag="d1")   # [d22|d11|d33]
                D1v = D1[:].rearrange("p (v l) -> p v l", v=3)
                nc.vector.tensor_add(D1v, SQv[:, :, 0], SQv[:, :, 1])
                nc.vector.tensor_add(D1v, D1v, SQv[:, :, 2])
                CP = ps.tile([128, 6 * R], dt, tag="cp")   # [v1*v2|v3*v1]
                nc.vector.tensor_mul(CP[:], V[:, 3 * R:9 * R], V[:, 0:6 * R])
                CPv = CP[:].rearrange("p (m c l) -> p m c l", m=2, c=3)
                DC = ps.tile([128, 2 * R], dt, tag="dc")   # [d12|d31]
                DCv = DC[:].rearrange("p (m l) -> p m l", m=2)
                nc.vector.tensor_add(DCv, CPv[:, :, 0], CPv[:, :, 1])
                nc.vector.tensor_add(DCv, DCv, CPv[:, :, 2])

                M = ps.tile([128, 2 * R], dt, tag="m")
                nc.vector.tensor_mul(M[:, 0:R], D1[:, 0:R], D1[:, R:2 * R])
                nc.vector.tensor_mul(M[:, R:2 * R], D1[:, 2 * R:3 * R],
                                     D1[:, R:2 * R])
                PSQ = ps.tile([128, 2 * R], dt, tag="psq")
                nc.scalar.activation(PSQ[:], DC[:], Act.Square)
                SG = ps.tile([128, 2 * R], dt, tag="sg")
                nc.scalar.activation(SG[:], DC[:], Act.Sign, bias=1e-35)
                S2 = ps.tile([128, 2 * R], dt, tag="s2")
                nc.vector.tensor_sub(S2[:], M[:], PSQ[:])
                LNIN = ps.tile([128, 5 * R], dt, tag="lnin")
                nc.vector.tensor_scalar_max(LNIN[:, 0:2 * R], S2[:], 1e-30)
                nc.vector.tensor_mul(LNIN[:, 2 * R:4 * R], DC[:], SG[:])
                nc.vector.tensor_scalar_max(
                    LNIN[:, 2 * R:4 * R], LNIN[:, 2 * R:4 * R], 1e-35)
                nc.vector.tensor_scalar_max(LNIN[:, 4 * R:5 * R],
                                            D1[:, R:2 * R], 1e-30)
                st1.append((b0, P, SG, LNIN))

            # -------- phase 2: all Ln -------------------------------------
            act_chain = []   # enforced execution order of Ln/Exp/Arctan
            for c in range(CH):
                b0, P, SG, LNIN = st1[c]
                LNO = ps.tile([128, 5 * R], dt, tag="lno")
                ln_i = nc.scalar.activation(LNO[:], LNIN[:], Act.Ln)
                act_chain.append(ln_i)
                st2.append((b0, P, SG, LNO, ln_i))

            # -------- phase 3: r/sign/|r| then all Exp --------------------
            st3 = []
            for c in range(CH):
                b0, P, SG, LNO, _ = st2[c]
                RT = ps.tile([128, 2 * R], dt, tag="rt")
                nc.vector.scalar_tensor_tensor(
                    RT[:], LNO[:, 0:2 * R], -0.5, LNO[:, 2 * R:4 * R],
                    op0=Alu.mult, op1=Alu.add)
                SR = ps.tile([128, 2 * R], dt, tag="sr")
                nc.scalar.activation(SR[:], RT[:], Act.Sign)
                ABSR = ps.tile([128, 2 * R], dt, tag="absr")
                nc.vector.tensor_mul(ABSR[:], RT[:], SR[:])
                EN = ps.tile([128, 2 * R], dt, tag="en")
                en_i = nc.scalar.activation(EN[:], ABSR[:], Act.Exp, scale=-1.0)
                F1 = ps.tile([128, R], dt, tag="f1")
                f1_i = nc.scalar.activation(F1[:], LNO[:, 4 * R:5 * R],
                                            Act.Exp, scale=0.5)
                act_chain.extend([en_i, f1_i])
                st3.append((b0, P, SG, SR, EN, F1, f1_i))

            # -------- phase 4: all Arctan + scoring -----------------------
            for c in range(CH):
                b0, P, SG, SR, EN, F1, _ = st3[c]
                TP = ps.tile([128, 2 * R], dt, tag="tp")
                tp_i = nc.scalar.activation(TP[:], EN[:], Act.Arctan)
                act_chain.append(tp_i)
                TB = ps.tile([128, 2 * R], dt, tag="tb")
                nc.vector.tensor_scalar(TB[:], TP[:], -1.0, np.pi / 4,
                                        op0=Alu.mult, op1=Alu.add)
                TC = ps.tile([128, 2 * R], dt, tag="tc")
                nc.gpsimd.tensor_mul(TC[:], SR[:], TB[:])
                AV = ps.tile([128, 2 * R], dt, tag="av")
                nc.gpsimd.tensor_mul(AV[:], SG[:], P[:, R:3 * R])
                W = ps.tile([128, 3 * R], dt, tag="w")
                nc.vector.tensor_mul(W[:, 0:R], F1[:], P[:, 3 * R:4 * R])
                nc.vector.scalar_tensor_tensor(
                    W[:, R:3 * R], TC[:], np.pi / 4, P[:, 4 * R:6 * R],
                    op0=Alu.add, op1=Alu.mult)
                U = ps.tile([128, 3 * R], dt, tag="u")
                nc.vector.tensor_sub(U[:, 0:R], W[:, 0:R], P[:, 0:R])
                nc.vector.tensor_sub(U[:, R:3 * R], W[:, R:3 * R], AV[:])
                Z = ps.tile([128, 3 * R], dt, tag="z")
                nc.gpsimd.tensor_mul(Z[:], U[:], U[:])
                ZC = ps.tile([128, 3 * R], dt, tag="zc")
                nc.vector.tensor_tensor(ZC[:], Z[:], P[:, 6 * R:9 * R],
                                        op=Alu.min)
                E = ps.tile([128, R], dt, tag="e")
                nc.gpsimd.tensor_add(E[:], ZC[:, 0:R], ZC[:, R:2 * R])
                nc.gpsimd.tensor_add(E[:], E[:], ZC[:, 2 * R:3 * R])
                nc.sync.dma_start(
                    O_t[b0:b0 + bc].rearrange("b c (k l) -> b c k l", k=KC),
                    E[:])

            # Chain the expensive ACT functions (walrus reloads the table on
            # every Ln/Exp/Arctan change): grouped order -> 3 loads total.
            for a, b_ in zip(act_chain[1:], act_chain[:-1]):
                _add_dep_helper(a.ins, b_.ins,
                                reason="ACT table-set grouping")
    return nc


def _get_program():
    if "nc" not in _PROGRAM_CACHE:
        nc = _build_program()
        nc.finalize()   # Bacc: register allocation / DCE / wait legalization
        _PROGRAM_CACHE["nc"] = nc
    return _PROGRAM_CACHE["nc"]


def _host_prep(atom_description, coords, mean, std, weight):
    ad = np.asarray(atom_description)
    coords = np.asarray(coords, dtype=np.float32)
    b, ch, rs, rn, an = (ad[:, i] for i in range(5))
    valid = (b >= 0) & (b < NB) & (ch >= 0) & (ch < MC) & (rs >= 0) & (rs < MR)

    def scat3(mask):
        A = np.full((NB, MC, MR, 3), PAD, np.float32)
        m = mask & valid
        A[b[m], ch[m], rs[m]] = coords[m]
        return A

    Narr, CAarr, Carr = scat3(an == 0), scat3(an == 1), scat3(an == 2)
    seq = np.full((NB, MC, MR), PAD_I, np.int64)
    m = (an == 1) & valid
    seq[b[m], ch[m], rs[m]] = rn[m]

    todo = ((Narr[:, :, 1:, 0] != PAD) & (Carr[:, :, :-1, 0] != PAD)
            & (CAarr[:, :, 1:, 0] != PAD) & (CAarr[:, :, :-1, 0] != PAD)
            & (seq[:, :, 1:] != PAD_I) & (seq[:, :, :-1] != PAD_I))
    sidx = np.clip(np.where(todo, seq[:, :, 1:], 0), 0, 19)

    w0 = float(np.asarray(weight).reshape(-1)[0])
    s_w = 1.0 - np.tanh(-w0)
    sq = np.sqrt(s_w)
    mu = np.asarray(mean, np.float64)
    sd = np.asarray(std, np.float64)
    q = 1.0 / (sd * np.sqrt(2.0))
    tab = np.empty((20, 9))
    tab[:, 0] = mu[:, 0] * q[:, 0] * sq
    tab[:, 1] = (np.pi / 2 - mu[:, 1]) * q[:, 1] * sq
    tab[:, 2] = (mu[:, 2] - np.pi / 2) * q[:, 2] * sq
    tab[:, 3:6] = q * sq
    tab[:, 6:9] = s_w * np.maximum(np.log(CL * q), 0.0)
    tab = tab.astype(np.float32)

    params = np.zeros((NB, MC, MR, 9), np.float32)
    params[:, :, 1:, :] = tab[sidx] * todo[..., None].astype(np.float32)
    # blocked coefficient-plane layout [NB, MC, KC, 9, R]
    pblk = np.ascontiguousarray(
        params.reshape(NB, MC, KC, R, 9).transpose(0, 1, 2, 4, 3))

    G = np.zeros((NB, MC, MR + 1, 9), np.float32)
    G[:, :, 1:, 0:3] = Narr
    G[:, :, 1:, 3:6] = CAarr
    G[:, :, 1:, 6:9] = Carr
    # blocked plane-contiguous with halo: GB[b,c,k,p,l] = G[b,c,k*R+l,p]
    GB = np.empty((NB, MC, KC, 9, R + 1), np.float32)
    for k in range(KC):
        GB[:, :, k] = G[:, :, k * R:k * R + R + 1, :].transpose(0, 1, 3, 2)
    return GB, pblk


def _install_ntff_hook():
    """The agent image's antenv lacks axon_hooks; synthesize it so
    trace=True can reach the terminal's NRT profiler (dev-only path)."""
    import sys, types
    if "antenv.axon_hooks" in sys.modules:
        return True
    try:
        import antenv
        mod = types.ModuleType("antenv.axon_hooks")
        mod._hook = None

        def set_axon_ntff_profile_hook(h):
            mod._hook = h

        def get_axon_ntff_profile_hook():
            return mod._hook

        mod.set_axon_ntff_profile_hook = set_axon_ntff_profile_hook
        mod.get_axon_ntff_profile_hook = get_axon_ntff_profile_hook
        sys.modules["antenv.axon_hooks"] = mod
        antenv.axon_hooks = mod
        from trn_agent_boot.trn_boot import _ntff_profile_via_ctypes
        mod._hook = _ntff_profile_via_ctypes("/opt/axon/libaxon_pjrt.so")
        return True
    except Exception as e:  # pragma: no cover - profiling is best-effort
        print(f"ntff hook install failed: {e}")
        return False


def kernel(**inputs):
    global LAST_RESULT
    from concourse.bass_utils import run_bass_kernel_spmd
    if TRACE:
        _install_ntff_hook()

    G, pblk = _host_prep(
        inputs["atom_description"], inputs["coords"],
        inputs["mean"], inputs["std"], inputs["weight"])

    nc = _get_program()
    in_maps = [
        {"g": np.ascontiguousarray(G[i * BPC:(i + 1) * BPC]),
         "pr": np.ascontiguousarray(pblk[i * BPC:(i + 1) * BPC])}
        for i in range(NCORES)
    ]
    res = run_bass_kernel_spmd(nc, in_maps, list(range(NCORES)), trace=TRACE)
    LAST_RESULT = res
    e = np.concatenate([res.results[i]["out"] for i in range(NCORES)], axis=0)
    e = e.reshape(NB, MC, MR)
    out = np.repeat(e[..., None], NALT, axis=-1)
    return np.ascontiguousarray(out.astype(np.float32))
